# revision 8
# baseline (speedup 1.0000x reference)
"""GroupedQueryAttention Trainium2 kernel (8 NeuronCores, tensor-parallel).

Sharding: heads across cores (4 q-heads / 1 kv-head per core).  Per core:
  - q/k/v projections for its heads (bf16 matmuls, fp32 PSUM accum)
  - interleaved RoPE applied via a rotation matmul + elementwise combine
  - causal flash-style attention in transposed layout:
      S^T[k,q] = K^T q  (row-packed pairs of heads, K=64 each),
      P = exp(S/8)  on ACT, row-sums via ones-augmented V (M=65 PV matmul)
  - AllToAll redistributes y from head-sharded to token-sharded
  - output projection for the core's 512-token slice, host concatenates.

Emission order interleaves batch-1 projections/RoPE after batch-0
attention so the PE queue has filler work while ACT runs the softmax
exps; ACT carries only exps, copies run on DVE, causal masks on GpSimd.

B=2, T=2048, DIM=2048, 32 q-heads / 8 kv-heads, head_dim 64.
"""
import os
import sys
import types

os.environ["JAX_PLATFORMS"] = ""  # axon PJRT must be allowed to register
if "/opt/trn_rl_repo" not in sys.path:
    sys.path.insert(0, "/opt/trn_rl_repo")

import numpy as np
import ml_dtypes

BF16_NP = ml_dtypes.bfloat16

B, T, DIM = 2, 2048, 2048
N_HEADS, N_KV_HEADS = 32, 8
HD = 64
CORES = 8
HLOC = N_HEADS // CORES      # 4 local q heads
PAIRS = HLOC // 2            # 2 head pairs
TOK = B * T
TQ = 512
TK = 128
NQT = T // TQ                # 4 q-tiles per batch
NKB = T // TK                # 16 key blocks per batch
TOK_PER_CORE = TOK // CORES  # 512


def _install_ntff_hook():
    import antenv
    if "antenv.axon_hooks" in sys.modules:
        return
    mod = types.ModuleType("antenv.axon_hooks")
    mod._hook = None
    def _set(h): mod._hook = h
    def _get(): return mod._hook
    mod.set_axon_ntff_profile_hook = _set
    mod.get_axon_ntff_profile_hook = _get
    sys.modules["antenv.axon_hooks"] = mod
    antenv.axon_hooks = mod
    try:
        from trn_agent_boot.trn_boot import _ntff_profile_via_ctypes
        _set(_ntff_profile_via_ctypes("/opt/axon/libaxon_pjrt.so"))
    except Exception:
        pass


def _build_nc():
    import concourse.mybir as mybir
    import concourse.tile as tile
    from concourse import bacc

    F32 = mybir.dt.float32
    BF16 = mybir.dt.bfloat16
    EXP = mybir.ActivationFunctionType.Exp

    nc = bacc.Bacc("TRN2", target_bir_lowering=False, debug=False,
                   num_devices=CORES)

    xT = nc.dram_tensor("xT", [DIM, TOK], BF16, kind="ExternalInput")
    wqkvT = nc.dram_tensor("wqkvT", [DIM, 384], BF16, kind="ExternalInput")
    woT = nc.dram_tensor("woT", [DIM, DIM], BF16, kind="ExternalInput")
    cdup = nc.dram_tensor("cdup", [128, T], BF16, kind="ExternalInput")
    sdup = nc.dram_tensor("sdup", [128, T], BF16, kind="ExternalInput")
    r2t = nc.dram_tensor("r2t", [128, 128], BF16, kind="ExternalInput")
    rt2 = nc.dram_tensor("rt2", [64, 128], BF16, kind="ExternalInput")
    iid = nc.dram_tensor("iid", [64, 128], BF16, kind="ExternalInput")
    ident = nc.dram_tensor("ident", [128, 128], BF16, kind="ExternalInput")
    tri = nc.dram_tensor("tri", [128, 128], BF16, kind="ExternalInput")
    ones64 = nc.dram_tensor("ones64", [1, 64], BF16, kind="ExternalInput")
    warm = nc.dram_tensor("warm", [8, 16], BF16, kind="ExternalInput")

    out = nc.dram_tensor("out", [TOK_PER_CORE, DIM], F32, kind="ExternalOutput")
    warm_o = nc.dram_tensor("warm_o", [8, 16], BF16, kind="ExternalOutput")

    rg = [list(range(CORES))]

    with tile.TileContext(nc) as tc:
        with (
            tc.tile_pool(name="res", bufs=1) as res,
            tc.tile_pool(name="pre", bufs=3) as pre,
            tc.tile_pool(name="xtp", bufs=4) as xtp,
            tc.tile_pool(name="sb", bufs=2) as sb,
            tc.tile_pool(name="psb", bufs=3, space="PSUM") as psb,
            tc.tile_pool(name="pss", bufs=2, space="PSUM") as pss,
            tc.tile_pool(name="dram", bufs=1, space="DRAM") as dram,
        ):
            w_sb = res.tile([128, 16 * 384], BF16, name="w_sb")
            cdup_sb = res.tile([128, T], BF16, name="cdup_sb")
            sdup_sb = res.tile([128, T], BF16, name="sdup_sb")
            r2t_sb = res.tile([128, 128], BF16, name="r2t_sb")
            rt2_sb = res.tile([64, 128], BF16, name="rt2_sb")
            iid_sb = res.tile([64, 128], BF16, name="iid_sb")
            ident_sb = res.tile([128, 128], BF16, name="ident_sb")
            tri_sb = res.tile([128, 128], BF16, name="tri_sb")
            ones_sb = res.tile([1, 64], BF16, name="ones_sb")
            qrope = [res.tile([128, TOK], BF16, name=f"qrope{s}") for s in range(2)]
            krope = res.tile([128, TOK], BF16, name="krope")
            vaug = res.tile([128, 2 * NKB * 65], BF16, name="vaug")
            wo_sb = res.tile([128, 16 * 2048], BF16, name="wo_sb")

            qpre = [pre.tile([128, TOK], BF16, name=f"qpre{s}", tag="pre")
                    for s in range(2)]
            kvpre = pre.tile([128, TOK], BF16, name="kvpre", tag="pre")

            warm_in = dram.tile([8, 16], BF16, name="warm_in")
            warm_out = dram.tile([8, 16], BF16, name="warm_out")
            a2a_in = [dram.tile([CORES, 128, TOK_PER_CORE], BF16, name=f"a2a_in{p}")
                      for p in range(PAIRS)]
            a2a_out = [dram.tile([CORES, 128, TOK_PER_CORE], BF16, name=f"a2a_out{p}")
                       for p in range(PAIRS)]

            # --- consts on scalar queue; w_sb slabs stream inside quarter 0
            nc.scalar.dma_start(out=cdup_sb[:], in_=cdup[:, :])
            nc.scalar.dma_start(out=sdup_sb[:], in_=sdup[:, :])
            nc.scalar.dma_start(out=r2t_sb[:], in_=r2t[:, :])
            nc.scalar.dma_start(out=rt2_sb[:], in_=rt2[:, :])
            nc.scalar.dma_start(out=iid_sb[:], in_=iid[:, :])
            nc.scalar.dma_start(out=ident_sb[:], in_=ident[:, :])
            nc.scalar.dma_start(out=tri_sb[:], in_=tri[:, :])
            nc.scalar.dma_start(out=ones_sb[:], in_=ones64[:, :])
            nc.gpsimd.memset(vaug[:], 1.0)
            # warm-up collective on the gpsimd queue only: absorbs cross-core
            # start skew without ever blocking the sync DMA queue.
            nc.gpsimd.dma_start(out=warm_in[:], in_=warm[:, :])
            nc.gpsimd.collective_compute(
                "AllToAll", mybir.AluOpType.bypass, replica_groups=rg,
                ins=[warm_in.opt()], outs=[warm_out.opt()])
            nc.gpsimd.dma_start(out=warm_o[:, :], in_=warm_out[:])

            def proj_quarter(nq):
                c0 = nq * 1024
                ps_q0 = psb.tile([128, 1024], F32, name="ps_q0", tag="big")
                ps_q1 = psb.tile([128, 1024], F32, name="ps_q1", tag="big")
                ps_kv0 = pss.tile([128, 512], F32, name="ps_kv0", tag="sm")
                ps_kv1 = pss.tile([128, 512], F32, name="ps_kv1", tag="sm")
                for kt in range(16):
                    if nq == 0:
                        nc.sync.dma_start(out=w_sb[:, kt * 384:(kt + 1) * 384],
                                          in_=wqkvT[kt * 128:(kt + 1) * 128, :])
                    xt = xtp.tile([128, 1024], BF16, name="xt", tag="xt")
                    nc.sync.dma_start(
                        out=xt[:], in_=xT[kt * 128:(kt + 1) * 128, c0:c0 + 1024])
                    st, sp = (kt == 0), (kt == 15)
                    w0 = w_sb[:, kt * 384:kt * 384 + 128]
                    w1 = w_sb[:, kt * 384 + 128:kt * 384 + 256]
                    w2 = w_sb[:, kt * 384 + 256:kt * 384 + 384]
                    nc.tensor.matmul(ps_q0[:, 0:512], w0, xt[:, 0:512], start=st, stop=sp)
                    nc.tensor.matmul(ps_q0[:, 512:1024], w0, xt[:, 512:1024], start=st, stop=sp)
                    nc.tensor.matmul(ps_q1[:, 0:512], w1, xt[:, 0:512], start=st, stop=sp)
                    nc.tensor.matmul(ps_q1[:, 512:1024], w1, xt[:, 512:1024], start=st, stop=sp)
                    nc.tensor.matmul(ps_kv0[:], w2, xt[:, 0:512], start=st, stop=sp)
                    nc.tensor.matmul(ps_kv1[:], w2, xt[:, 512:1024], start=st, stop=sp)
                nc.vector.tensor_copy(qpre[0][:, c0:c0 + 1024], ps_q0[:])
                nc.vector.tensor_copy(qpre[1][:, c0:c0 + 1024], ps_q1[:])
                nc.vector.tensor_copy(kvpre[:, c0:c0 + 512], ps_kv0[:])
                nc.vector.tensor_copy(kvpre[:, c0 + 512:c0 + 1024], ps_kv1[:])

            def rope_batch(b):
                for ci in range(4):
                    c = b * 4 + ci
                    gcols = slice(c * 512, (c + 1) * 512)
                    pcols = slice(ci * 512, (ci + 1) * 512)
                    rot_ps = pss.tile([128, 512], F32, name="rot_ps", tag="sm")
                    kdup_ps = pss.tile([128, 512], F32, name="kdup_ps", tag="sm")
                    nc.tensor.matmul(rot_ps[:], rt2_sb[:], kvpre[0:64, gcols])
                    nc.tensor.matmul(kdup_ps[:], iid_sb[:], kvpre[0:64, gcols])
                    rot_sb = sb.tile([128, 512], BF16, name="rot_sb", tag="rope", bufs=4)
                    kdup_sb = sb.tile([128, 512], BF16, name="kdup_sb", tag="rope", bufs=4)
                    nc.scalar.copy(rot_sb[:], rot_ps[:])
                    nc.scalar.copy(kdup_sb[:], kdup_ps[:])
                    t1k = sb.tile([128, 512], BF16, name="t1k", tag="rope", bufs=4)
                    nc.vector.tensor_mul(t1k[:], kdup_sb[:], cdup_sb[:, pcols])
                    nc.vector.tensor_mul(rot_sb[:], rot_sb[:], sdup_sb[:, pcols])
                    nc.vector.tensor_add(krope[:, gcols], t1k[:], rot_sb[:])
                    for j in range(4):
                        kb = ci * 4 + j
                        col = c * 512 + j * 128
                        vt_ps = pss.tile([128, 64], BF16, name="vt_ps", tag="sm")
                        nc.tensor.transpose(
                            vt_ps[:], kvpre[64:128, col:col + 128],
                            ident_sb[64:128, 64:128])
                        va = (b * NKB + kb) * 65
                        nc.vector.tensor_copy(vaug[:, va:va + 64], vt_ps[:])
                for s in range(2):
                    for ci in range(4):
                        c = b * 4 + ci
                        gcols = slice(c * 512, (c + 1) * 512)
                        pcols = slice(ci * 512, (ci + 1) * 512)
                        rq_ps = pss.tile([128, 512], F32, name="rq_ps", tag="sm")
                        nc.tensor.matmul(rq_ps[:], r2t_sb[:], qpre[s][:, gcols])
                        rq_sb = sb.tile([128, 512], BF16, name="rq_sb", tag="rope", bufs=4)
                        nc.scalar.copy(rq_sb[:], rq_ps[:])
                        t1q = sb.tile([128, 512], BF16, name="t1q", tag="rope", bufs=4)
                        nc.vector.tensor_mul(t1q[:], qpre[s][:, gcols], cdup_sb[:, pcols])
                        nc.vector.tensor_mul(rq_sb[:], rq_sb[:], sdup_sb[:, pcols])
                        nc.vector.tensor_add(qrope[s][:, gcols], t1q[:], rq_sb[:])

            def attn(p, b):
                bcol = b * T
                for qt in range(NQT):
                    q0 = bcol + qt * TQ
                    nkb = 4 * qt + 4
                    ypair = psb.tile([128, 1024], F32, name="ypair", tag="big")
                    for kb in range(nkb):
                        r = kb - 4 * qt
                        lo = 128 * r if r >= 0 else 0
                        k0 = bcol + kb * TK
                        st_ps = psb.tile([128, 1024], F32, name="st_ps", tag="big")
                        pt = sb.tile([128, 1024], BF16, name="pt", tag="pt", bufs=3)
                        for e in range(2):
                            nc.tensor.matmul(
                                st_ps[:, e * 512 + lo:e * 512 + 512],
                                krope[64 * e:64 * e + 64, k0:k0 + 128],
                                qrope[p][64 * e:64 * e + 64, q0 + lo:q0 + 512])
                        if r <= 0:
                            nc.scalar.activation(
                                pt[:, 0:1024], st_ps[:, 0:1024], EXP, scale=0.125)
                            if r == 0:
                                for e in range(2):
                                    nc.vector.tensor_mul(
                                        pt[:, e * 512:e * 512 + 128],
                                        pt[:, e * 512:e * 512 + 128],
                                        tri_sb[:])
                        else:
                            for e in range(2):
                                nc.scalar.activation(
                                    pt[:, e * 512 + lo:e * 512 + 512],
                                    st_ps[:, e * 512 + lo:e * 512 + 512],
                                    EXP, scale=0.125)
                                nc.vector.tensor_mul(
                                    pt[:, e * 512 + lo:e * 512 + lo + 128],
                                    pt[:, e * 512 + lo:e * 512 + lo + 128],
                                    tri_sb[:])
                        va = (b * NKB + kb) * 65
                        for e in range(2):
                            nc.tensor.matmul(
                                ypair[0:65, e * 512 + lo:e * 512 + 512],
                                vaug[:, va:va + 65],
                                pt[:, e * 512 + lo:e * 512 + 512],
                                start=(kb == 0), stop=(kb == nkb - 1))
                    shard = b * NQT + qt
                    for e in range(2):
                        lrow = sb.tile([1, 512], F32, name="lrow", tag="lin0", bufs=3)
                        nc.vector.tensor_copy(lrow[:], ypair[64:65, e * 512:e * 512 + 512])
                        linv = sb.tile([1, 512], F32, name="linv", tag="lin", bufs=3)
                        nc.vector.reciprocal_approx_fast(linv[:], lrow[:])
                        linb = sb.tile([1, 512], BF16, name="linb", tag="lin2", bufs=3)
                        nc.gpsimd.tensor_copy(linb[:], linv[:])
                        bc_sb = sb.tile([64, 512], BF16, name="bc_sb", tag="ep", bufs=4)
                        nc.gpsimd.partition_broadcast(bc_sb[:], linb[:])
                        y_sb = sb.tile([64, 512], BF16, name="y_sb", tag="ep", bufs=4)
                        nc.vector.tensor_mul(
                            y_sb[:], ypair[0:64, e * 512:e * 512 + 512], bc_sb[:])
                        nc.sync.dma_start(
                            out=a2a_in[p][shard, e * 64:(e + 1) * 64, :],
                            in_=y_sb[:])

            # ---- emission: b0 pipeline, attention gap-filled by b1 prep ----
            proj_quarter(0)
            proj_quarter(1)
            rope_batch(0)
            attn(0, 0)
            proj_quarter(2)          # fills PE gaps while ACT runs b0 exps
            proj_quarter(3)
            rope_batch(1)
            # wo slabs: stream during attention (scalar queue is idle-ish)
            for kt in range(16):
                nc.scalar.dma_start(out=wo_sb[:, kt * 2048:(kt + 1) * 2048],
                                    in_=woT[kt * 128:(kt + 1) * 128, :])
            attn(0, 1)
            nc.gpsimd.collective_compute(
                "AllToAll", mybir.AluOpType.bypass, replica_groups=rg,
                ins=[a2a_in[0].opt()], outs=[a2a_out[0].opt()])
            ysl = [pre.tile([128, 8 * 512], BF16, name=f"ysl{i}", tag="pre")
                   for i in range(2)]
            attn(1, 0)
            for kt in range(0, 16, 2):   # pair-0 slabs (a2a #0 done long ago)
                nc.gpsimd.dma_start(
                    out=ysl[kt // 8][:, (kt % 8) * 512:(kt % 8) * 512 + 512],
                    in_=a2a_out[0][kt // 2, :, :])
            attn(1, 1)
            nc.gpsimd.collective_compute(
                "AllToAll", mybir.AluOpType.bypass, replica_groups=rg,
                ins=[a2a_in[1].opt()], outs=[a2a_out[1].opt()])
            for kt in range(1, 16, 2):
                nc.gpsimd.dma_start(
                    out=ysl[kt // 8][:, (kt % 8) * 512:(kt % 8) * 512 + 512],
                    in_=a2a_out[1][kt // 2, :, :])

            # ---- out projection; even kt first so it overlaps a2a #1 ----
            kts = list(range(0, 16, 2)) + list(range(1, 16, 2))
            for mt in range(4):
                op0 = psb.tile([128, 1024], F32, name="op0", tag="big")
                op1 = psb.tile([128, 1024], F32, name="op1", tag="big")
                for idx, kt in enumerate(kts):
                    st, sp = (idx == 0), (idx == 15)
                    col = (kt % 8) * 512 + mt * 128
                    lhs = ysl[kt // 8][:, col:col + 128]
                    wcol = kt * 2048
                    nc.tensor.matmul(op0[:, 0:512], lhs, wo_sb[:, wcol:wcol + 512], start=st, stop=sp)
                    nc.tensor.matmul(op0[:, 512:1024], lhs, wo_sb[:, wcol + 512:wcol + 1024], start=st, stop=sp)
                    nc.tensor.matmul(op1[:, 0:512], lhs, wo_sb[:, wcol + 1024:wcol + 1536], start=st, stop=sp)
                    nc.tensor.matmul(op1[:, 512:1024], lhs, wo_sb[:, wcol + 1536:wcol + 2048], start=st, stop=sp)
                for half, op in ((0, op0), (1, op1)):
                    osb = sb.tile([128, 1024], F32, name="osb", tag="osb", bufs=2)
                    nc.vector.tensor_copy(osb[:], op[:])
                    nc.sync.dma_start(
                        out=out[mt * 128:(mt + 1) * 128, half * 1024:(half + 1) * 1024],
                        in_=osb[:])

    nc.compile()
    return nc


_NC_CACHE = {}


def _get_nc():
    if "nc" not in _NC_CACHE:
        _NC_CACHE["nc"] = _build_nc()
    return _NC_CACHE["nc"]


def _host_prep(x, cos, sin, wq, wk, wv, wo):
    x = np.asarray(x, dtype=np.float32)
    cos = np.asarray(cos, dtype=np.float32)[:T]
    sin = np.asarray(sin, dtype=np.float32)[:T]
    wq = np.asarray(wq, dtype=np.float32)
    wk = np.asarray(wk, dtype=np.float32)
    wv = np.asarray(wv, dtype=np.float32)
    wo = np.asarray(wo, dtype=np.float32)

    xT = np.ascontiguousarray(x.reshape(TOK, DIM).T).astype(BF16_NP)
    woT = np.ascontiguousarray(wo.T).astype(BF16_NP)

    pair = np.arange(64) // 2
    cd = cos[:, pair].T
    sd = sin[:, pair].T
    cdup = np.ascontiguousarray(np.concatenate([cd, cd], axis=0)).astype(BF16_NP)
    sdup = np.ascontiguousarray(np.concatenate([sd, sd], axis=0)).astype(BF16_NP)

    R = np.zeros((64, 64), dtype=np.float32)
    for j in range(32):
        R[2 * j, 2 * j + 1] = -1.0
        R[2 * j + 1, 2 * j] = 1.0
    RT = R.T
    r2t = np.zeros((128, 128), dtype=np.float32)
    r2t[0:64, 0:64] = RT
    r2t[64:128, 64:128] = RT
    rt2 = np.concatenate([RT, RT], axis=1)
    iid = np.concatenate([np.eye(64, dtype=np.float32)] * 2, axis=1)
    ident = np.eye(128, dtype=np.float32)
    kp = np.arange(128)[:, None]
    cc = np.arange(128)[None, :]
    tri = (kp <= cc).astype(np.float32)

    consts = {
        "xT": xT, "woT": woT, "cdup": cdup, "sdup": sdup,
        "r2t": r2t.astype(BF16_NP),
        "rt2": np.ascontiguousarray(rt2).astype(BF16_NP),
        "iid": np.ascontiguousarray(iid).astype(BF16_NP),
        "ident": ident.astype(BF16_NP), "tri": tri.astype(BF16_NP),
        "ones64": np.ones((1, 64), dtype=np.float32).astype(BF16_NP),
        "warm": np.zeros((8, 16), dtype=np.float32).astype(BF16_NP),
    }
    in_maps = []
    for g in range(CORES):
        wqkv = np.concatenate([
            wq[g * 256:(g + 1) * 256],
            wk[g * 64:(g + 1) * 64],
            wv[g * 64:(g + 1) * 64],
        ], axis=0)
        m = dict(consts)
        m["wqkvT"] = np.ascontiguousarray(wqkv.T).astype(BF16_NP)
        in_maps.append(m)
    return in_maps


def run(inputs, trace=False):
    _install_ntff_hook()
    from concourse import bass_utils
    nc = _get_nc()
    in_maps = _host_prep(**inputs)
    res = bass_utils.run_bass_kernel_spmd(
        nc, in_maps, core_ids=list(range(CORES)), trace=trace)
    full = np.empty((TOK, DIM), dtype=np.float32)
    for g in range(CORES):
        full[g * TOK_PER_CORE:(g + 1) * TOK_PER_CORE] = res.results[g]["out"]
    return full.reshape(B, T, DIM), res.exec_time_ns


def kernel(x, cos, sin, wq, wk, wv, wo):
    out, _ = run(dict(x=x, cos=cos, sin=sin, wq=wq, wk=wk, wv=wv, wo=wo),
                 trace=False)
    return out


# revision 12
# speedup vs baseline: 1.4106x; 1.4106x over previous
"""GroupedQueryAttention Trainium2 kernel (8 NeuronCores, tensor-parallel).

Sharding: heads across cores (4 q-heads / 1 kv-head per core).  Per core:
  - q/k/v projections for its heads (bf16 matmuls, fp32 PSUM accum)
  - interleaved RoPE applied via a rotation matmul + elementwise combine
  - causal flash-style attention in transposed layout:
      S^T[k,q] = K^T q  (row-packed pairs of heads, K=64 each),
      P = exp(S/8)  on ACT, row-sums via ones-augmented V (M=65 PV matmul)
  - AllToAll redistributes y from head-sharded to token-sharded
  - output projection for the core's 512-token slice, host concatenates.

Emission order interleaves batch-1 projections/RoPE after batch-0
attention so the PE queue has filler work while ACT runs the softmax
exps; ACT carries only exps, copies run on DVE, causal masks on GpSimd.

B=2, T=2048, DIM=2048, 32 q-heads / 8 kv-heads, head_dim 64.
"""
import os
import sys
import types

os.environ["JAX_PLATFORMS"] = ""  # axon PJRT must be allowed to register
if "/opt/trn_rl_repo" not in sys.path:
    sys.path.insert(0, "/opt/trn_rl_repo")

import numpy as np
import ml_dtypes

BF16_NP = ml_dtypes.bfloat16

B, T, DIM = 2, 2048, 2048
N_HEADS, N_KV_HEADS = 32, 8
HD = 64
CORES = 8
HLOC = N_HEADS // CORES      # 4 local q heads
PAIRS = HLOC // 2            # 2 head pairs
TOK = B * T
TQ = 512
TK = 128
NQT = T // TQ                # 4 q-tiles per batch
NKB = T // TK                # 16 key blocks per batch
TOK_PER_CORE = TOK // CORES  # 512


def _install_ntff_hook():
    import antenv
    if "antenv.axon_hooks" in sys.modules:
        return
    mod = types.ModuleType("antenv.axon_hooks")
    mod._hook = None
    def _set(h): mod._hook = h
    def _get(): return mod._hook
    mod.set_axon_ntff_profile_hook = _set
    mod.get_axon_ntff_profile_hook = _get
    sys.modules["antenv.axon_hooks"] = mod
    antenv.axon_hooks = mod
    try:
        from trn_agent_boot.trn_boot import _ntff_profile_via_ctypes
        _set(_ntff_profile_via_ctypes("/opt/axon/libaxon_pjrt.so"))
    except Exception:
        pass


def _build_nc():
    import concourse.mybir as mybir
    import concourse.tile as tile
    from concourse import bacc

    F32 = mybir.dt.float32
    BF16 = mybir.dt.bfloat16
    EXP = mybir.ActivationFunctionType.Exp

    nc = bacc.Bacc("TRN2", target_bir_lowering=False, debug=False,
                   num_devices=CORES)

    xT = nc.dram_tensor("xT", [DIM, TOK], BF16, kind="ExternalInput")
    wqkvT = nc.dram_tensor("wqkvT", [DIM, 384], BF16, kind="ExternalInput")
    woT = nc.dram_tensor("woT", [DIM, DIM], BF16, kind="ExternalInput")
    cdup = nc.dram_tensor("cdup", [128, T], BF16, kind="ExternalInput")
    sdup = nc.dram_tensor("sdup", [128, T], BF16, kind="ExternalInput")
    r2t = nc.dram_tensor("r2t", [128, 128], BF16, kind="ExternalInput")
    rt2 = nc.dram_tensor("rt2", [64, 128], BF16, kind="ExternalInput")
    iid = nc.dram_tensor("iid", [64, 128], BF16, kind="ExternalInput")
    ident = nc.dram_tensor("ident", [128, 128], BF16, kind="ExternalInput")
    tri = nc.dram_tensor("tri", [128, 128], BF16, kind="ExternalInput")
    ones64 = nc.dram_tensor("ones64", [1, 64], BF16, kind="ExternalInput")
    warm = nc.dram_tensor("warm", [8, 16], BF16, kind="ExternalInput")

    out = nc.dram_tensor("out", [TOK_PER_CORE, DIM], F32, kind="ExternalOutput")
    warm_o = nc.dram_tensor("warm_o", [8, 16], BF16, kind="ExternalOutput")

    rg = [list(range(CORES))]

    with tile.TileContext(nc) as tc:
        with (
            tc.tile_pool(name="res", bufs=1) as res,
            tc.tile_pool(name="pre", bufs=3) as pre,
            tc.tile_pool(name="xtp", bufs=4) as xtp,
            tc.tile_pool(name="sb", bufs=2) as sb,
            tc.tile_pool(name="psb", bufs=2, space="PSUM") as psb,
            tc.tile_pool(name="pss", bufs=4, space="PSUM") as pss,
            tc.tile_pool(name="dram", bufs=1, space="DRAM") as dram,
        ):
            w_sb = res.tile([128, 16 * 384], BF16, name="w_sb")
            cdup_sb = res.tile([128, T], BF16, name="cdup_sb")
            sdup_sb = res.tile([128, T], BF16, name="sdup_sb")
            r2t_sb = res.tile([128, 128], BF16, name="r2t_sb")
            rt2_sb = res.tile([64, 128], BF16, name="rt2_sb")
            iid_sb = res.tile([64, 128], BF16, name="iid_sb")
            ident_sb = res.tile([128, 128], BF16, name="ident_sb")
            tri_sb = res.tile([128, 128], BF16, name="tri_sb")
            ones_sb = res.tile([1, 64], BF16, name="ones_sb")
            qrope = [res.tile([128, TOK], BF16, name=f"qrope{s}") for s in range(2)]
            krope = res.tile([128, TOK], BF16, name="krope")
            vaug = res.tile([128, 2 * NKB * 65], BF16, name="vaug")
            wo_sb = res.tile([128, 16 * 2048], BF16, name="wo_sb")

            qpre = [pre.tile([128, TOK], BF16, name=f"qpre{s}", tag="pre")
                    for s in range(2)]
            kvpre = pre.tile([128, TOK], BF16, name="kvpre", tag="pre")

            warm_in = dram.tile([8, 16], BF16, name="warm_in")
            warm_out = dram.tile([8, 16], BF16, name="warm_out")
            a2a_in = [dram.tile([CORES, 128, TOK_PER_CORE], BF16, name=f"a2a_in{p}")
                      for p in range(PAIRS)]
            a2a_out = [dram.tile([CORES, 128, TOK_PER_CORE], BF16, name=f"a2a_out{p}")
                       for p in range(PAIRS)]

            # --- consts on scalar queue; w_sb slabs stream inside quarter 0
            nc.scalar.dma_start(out=cdup_sb[:], in_=cdup[:, :])
            nc.scalar.dma_start(out=sdup_sb[:], in_=sdup[:, :])
            nc.scalar.dma_start(out=r2t_sb[:], in_=r2t[:, :])
            nc.scalar.dma_start(out=rt2_sb[:], in_=rt2[:, :])
            nc.scalar.dma_start(out=iid_sb[:], in_=iid[:, :])
            nc.scalar.dma_start(out=ident_sb[:], in_=ident[:, :])
            nc.scalar.dma_start(out=tri_sb[:], in_=tri[:, :])
            nc.scalar.dma_start(out=ones_sb[:], in_=ones64[:, :])
            nc.gpsimd.memset(vaug[:], 1.0)
            # warm-up collective on the gpsimd queue only: absorbs cross-core
            # start skew without ever blocking the sync DMA queue.
            nc.gpsimd.dma_start(out=warm_in[:], in_=warm[:, :])
            nc.gpsimd.collective_compute(
                "AllToAll", mybir.AluOpType.bypass, replica_groups=rg,
                ins=[warm_in.opt()], outs=[warm_out.opt()])
            nc.gpsimd.dma_start(out=warm_o[:, :], in_=warm_out[:])

            def proj_quarter(nq):
                c0 = nq * 1024
                ps_q0 = psb.tile([128, 1024], F32, name="ps_q0", tag="big")
                ps_q1 = psb.tile([128, 1024], F32, name="ps_q1", tag="big")
                ps_kv0 = pss.tile([128, 512], F32, name="ps_kv0", tag="sm")
                ps_kv1 = pss.tile([128, 512], F32, name="ps_kv1", tag="sm")
                for kt in range(16):
                    if nq == 0:
                        nc.sync.dma_start(out=w_sb[:, kt * 384:(kt + 1) * 384],
                                          in_=wqkvT[kt * 128:(kt + 1) * 128, :])
                    xt = xtp.tile([128, 1024], BF16, name="xt", tag="xt")
                    nc.sync.dma_start(
                        out=xt[:], in_=xT[kt * 128:(kt + 1) * 128, c0:c0 + 1024])
                    st, sp = (kt == 0), (kt == 15)
                    w0 = w_sb[:, kt * 384:kt * 384 + 128]
                    w1 = w_sb[:, kt * 384 + 128:kt * 384 + 256]
                    w2 = w_sb[:, kt * 384 + 256:kt * 384 + 384]
                    nc.tensor.matmul(ps_q0[:, 0:512], w0, xt[:, 0:512], start=st, stop=sp)
                    nc.tensor.matmul(ps_q0[:, 512:1024], w0, xt[:, 512:1024], start=st, stop=sp)
                    nc.tensor.matmul(ps_q1[:, 0:512], w1, xt[:, 0:512], start=st, stop=sp)
                    nc.tensor.matmul(ps_q1[:, 512:1024], w1, xt[:, 512:1024], start=st, stop=sp)
                    nc.tensor.matmul(ps_kv0[:], w2, xt[:, 0:512], start=st, stop=sp)
                    nc.tensor.matmul(ps_kv1[:], w2, xt[:, 512:1024], start=st, stop=sp)
                nc.vector.tensor_copy(qpre[0][:, c0:c0 + 1024], ps_q0[:])
                nc.vector.tensor_copy(qpre[1][:, c0:c0 + 1024], ps_q1[:])
                nc.vector.tensor_copy(kvpre[:, c0:c0 + 512], ps_kv0[:])
                nc.vector.tensor_copy(kvpre[:, c0 + 512:c0 + 1024], ps_kv1[:])

            def rope_batch(b):
                for ci in range(4):
                    c = b * 4 + ci
                    gcols = slice(c * 512, (c + 1) * 512)
                    pcols = slice(ci * 512, (ci + 1) * 512)
                    rot_ps = pss.tile([128, 512], F32, name="rot_ps", tag="sm")
                    kdup_ps = pss.tile([128, 512], F32, name="kdup_ps", tag="sm")
                    nc.tensor.matmul(rot_ps[:], rt2_sb[:], kvpre[0:64, gcols])
                    nc.tensor.matmul(kdup_ps[:], iid_sb[:], kvpre[0:64, gcols])
                    rot_sb = sb.tile([128, 512], BF16, name="rot_sb", tag="rope", bufs=4)
                    kdup_sb = sb.tile([128, 512], BF16, name="kdup_sb", tag="rope", bufs=4)
                    nc.scalar.copy(rot_sb[:], rot_ps[:])
                    nc.scalar.copy(kdup_sb[:], kdup_ps[:])
                    t1k = sb.tile([128, 512], BF16, name="t1k", tag="rope", bufs=4)
                    nc.vector.tensor_mul(t1k[:], kdup_sb[:], cdup_sb[:, pcols])
                    nc.vector.tensor_mul(rot_sb[:], rot_sb[:], sdup_sb[:, pcols])
                    nc.vector.tensor_add(krope[:, gcols], t1k[:], rot_sb[:])
                    for j in range(4):
                        kb = ci * 4 + j
                        col = c * 512 + j * 128
                        vt_ps = pss.tile([128, 64], BF16, name="vt_ps", tag="sm")
                        nc.tensor.transpose(
                            vt_ps[:], kvpre[64:128, col:col + 128],
                            ident_sb[64:128, 64:128])
                        va = (b * NKB + kb) * 65
                        nc.vector.tensor_copy(vaug[:, va:va + 64], vt_ps[:])
                for s in range(2):
                    for ci in range(4):
                        c = b * 4 + ci
                        gcols = slice(c * 512, (c + 1) * 512)
                        pcols = slice(ci * 512, (ci + 1) * 512)
                        rq_ps = pss.tile([128, 512], F32, name="rq_ps", tag="sm")
                        nc.tensor.matmul(rq_ps[:], r2t_sb[:], qpre[s][:, gcols])
                        rq_sb = sb.tile([128, 512], BF16, name="rq_sb", tag="rope", bufs=4)
                        nc.scalar.copy(rq_sb[:], rq_ps[:])
                        t1q = sb.tile([128, 512], BF16, name="t1q", tag="rope", bufs=4)
                        nc.vector.tensor_mul(t1q[:], qpre[s][:, gcols], cdup_sb[:, pcols])
                        nc.vector.tensor_mul(rq_sb[:], rq_sb[:], sdup_sb[:, pcols])
                        nc.vector.tensor_add(qrope[s][:, gcols], t1q[:], rq_sb[:])

            def attn(p, b):
                bcol = b * T
                for qt in range(NQT):
                    q0 = bcol + qt * TQ
                    nkb = 4 * qt + 4
                    yac = [pss.tile([65, 512], F32, name=f"yac{e}", tag="sm")
                           for e in range(2)]

                    def do_pv(kb, lo, pt):
                        va = (b * NKB + kb) * 65
                        for e in range(2):
                            nc.tensor.matmul(
                                yac[e][0:65, lo:512],
                                vaug[:, va:va + 65],
                                pt[:, e * 512 + lo:e * 512 + 512],
                                start=(kb == 0), stop=(kb == nkb - 1))

                    pend = None   # PV trails ST by one slot: exp latency hides
                    for kb in range(nkb):
                        r = kb - 4 * qt
                        lo = 128 * r if r >= 0 else 0
                        k0 = bcol + kb * TK
                        st_ps = psb.tile([128, 1024], F32, name="st_ps", tag="big")
                        pt = sb.tile([128, 1024], BF16, name="pt", tag="pt", bufs=3)
                        for e in range(2):
                            nc.tensor.matmul(
                                st_ps[:, e * 512 + lo:e * 512 + 512],
                                krope[64 * e:64 * e + 64, k0:k0 + 128],
                                qrope[p][64 * e:64 * e + 64, q0 + lo:q0 + 512])
                        if pend is not None:
                            do_pv(*pend)
                        if r <= 0:
                            nc.scalar.activation(
                                pt[:, 0:1024], st_ps[:, 0:1024], EXP, scale=0.125)
                            if r == 0:
                                for e in range(2):
                                    nc.vector.tensor_mul(
                                        pt[:, e * 512:e * 512 + 128],
                                        pt[:, e * 512:e * 512 + 128],
                                        tri_sb[:])
                        else:
                            for e in range(2):
                                nc.scalar.activation(
                                    pt[:, e * 512 + lo:e * 512 + 512],
                                    st_ps[:, e * 512 + lo:e * 512 + 512],
                                    EXP, scale=0.125)
                                nc.vector.tensor_mul(
                                    pt[:, e * 512 + lo:e * 512 + lo + 128],
                                    pt[:, e * 512 + lo:e * 512 + lo + 128],
                                    tri_sb[:])
                        pend = (kb, lo, pt)
                    do_pv(*pend)
                    shard = b * NQT + qt
                    for e in range(2):
                        lrow = sb.tile([1, 512], F32, name="lrow", tag="lin0", bufs=3)
                        nc.vector.tensor_copy(lrow[:], yac[e][64:65, 0:512])
                        linv = sb.tile([1, 512], F32, name="linv", tag="lin", bufs=3)
                        nc.vector.reciprocal_approx_fast(linv[:], lrow[:])
                        linb = sb.tile([1, 512], BF16, name="linb", tag="lin2", bufs=3)
                        nc.gpsimd.tensor_copy(linb[:], linv[:])
                        bc_sb = sb.tile([64, 512], BF16, name="bc_sb", tag="ep", bufs=4)
                        nc.gpsimd.partition_broadcast(bc_sb[:], linb[:])
                        y_sb = sb.tile([64, 512], BF16, name="y_sb", tag="ep", bufs=4)
                        nc.vector.tensor_mul(y_sb[:], yac[e][0:64, 0:512], bc_sb[:])
                        nc.sync.dma_start(
                            out=a2a_in[p][shard, e * 64:(e + 1) * 64, :],
                            in_=y_sb[:])

            # ---- emission: b0 pipeline, attention gap-filled by b1 prep ----
            proj_quarter(0)
            proj_quarter(1)
            rope_batch(0)
            attn(0, 0)
            proj_quarter(2)          # fills PE gaps while ACT runs b0 exps
            proj_quarter(3)
            rope_batch(1)
            # wo slabs: stream during attention (scalar queue is idle-ish)
            for kt in range(16):
                nc.scalar.dma_start(out=wo_sb[:, kt * 2048:(kt + 1) * 2048],
                                    in_=woT[kt * 128:(kt + 1) * 128, :])
            attn(0, 1)
            nc.gpsimd.collective_compute(
                "AllToAll", mybir.AluOpType.bypass, replica_groups=rg,
                ins=[a2a_in[0].opt()], outs=[a2a_out[0].opt()])
            ysl = [pre.tile([128, 8 * 512], BF16, name=f"ysl{i}", tag="pre")
                   for i in range(2)]
            attn(1, 0)
            for kt in range(0, 16, 2):   # pair-0 slabs (a2a #0 done long ago)
                nc.gpsimd.dma_start(
                    out=ysl[kt // 8][:, (kt % 8) * 512:(kt % 8) * 512 + 512],
                    in_=a2a_out[0][kt // 2, :, :])
            attn(1, 1)
            nc.gpsimd.collective_compute(
                "AllToAll", mybir.AluOpType.bypass, replica_groups=rg,
                ins=[a2a_in[1].opt()], outs=[a2a_out[1].opt()])
            for kt in range(1, 16, 2):
                nc.gpsimd.dma_start(
                    out=ysl[kt // 8][:, (kt % 8) * 512:(kt % 8) * 512 + 512],
                    in_=a2a_out[1][kt // 2, :, :])

            # ---- out projection, two passes: even kt (a2a #0 data) runs while
            # a2a #1 is in flight, partials staged in SBUF; odd kt added after.
            parts = []
            for mt in range(4):
                op0 = psb.tile([128, 1024], F32, name="op0", tag="big")
                op1 = psb.tile([128, 1024], F32, name="op1", tag="big")
                for idx, kt in enumerate(range(0, 16, 2)):
                    st, sp = (idx == 0), (idx == 7)
                    col = (kt % 8) * 512 + mt * 128
                    lhs = ysl[kt // 8][:, col:col + 128]
                    wcol = kt * 2048
                    nc.tensor.matmul(op0[:, 0:512], lhs, wo_sb[:, wcol:wcol + 512], start=st, stop=sp)
                    nc.tensor.matmul(op0[:, 512:1024], lhs, wo_sb[:, wcol + 512:wcol + 1024], start=st, stop=sp)
                    nc.tensor.matmul(op1[:, 0:512], lhs, wo_sb[:, wcol + 1024:wcol + 1536], start=st, stop=sp)
                    nc.tensor.matmul(op1[:, 512:1024], lhs, wo_sb[:, wcol + 1536:wcol + 2048], start=st, stop=sp)
                part = sb.tile([128, 2048], BF16, name=f"part{mt}", tag=f"part{mt}", bufs=1)
                nc.vector.tensor_copy(part[:, 0:1024], op0[:])
                nc.vector.tensor_copy(part[:, 1024:2048], op1[:])
                parts.append(part)
            for mt in range(4):
                op0 = psb.tile([128, 1024], F32, name="op0b", tag="big")
                op1 = psb.tile([128, 1024], F32, name="op1b", tag="big")
                for idx, kt in enumerate(range(1, 16, 2)):
                    st, sp = (idx == 0), (idx == 7)
                    col = (kt % 8) * 512 + mt * 128
                    lhs = ysl[kt // 8][:, col:col + 128]
                    wcol = kt * 2048
                    nc.tensor.matmul(op0[:, 0:512], lhs, wo_sb[:, wcol:wcol + 512], start=st, stop=sp)
                    nc.tensor.matmul(op0[:, 512:1024], lhs, wo_sb[:, wcol + 512:wcol + 1024], start=st, stop=sp)
                    nc.tensor.matmul(op1[:, 0:512], lhs, wo_sb[:, wcol + 1024:wcol + 1536], start=st, stop=sp)
                    nc.tensor.matmul(op1[:, 512:1024], lhs, wo_sb[:, wcol + 1536:wcol + 2048], start=st, stop=sp)
                for half, op in ((0, op0), (1, op1)):
                    osb = sb.tile([128, 1024], F32, name="osb", tag="osb", bufs=2)
                    nc.vector.tensor_add(
                        osb[:], op[:], parts[mt][:, half * 1024:(half + 1) * 1024])
                    nc.sync.dma_start(
                        out=out[mt * 128:(mt + 1) * 128, half * 1024:(half + 1) * 1024],
                        in_=osb[:])

    nc.compile()
    return nc


_NC_CACHE = {}


def _get_nc():
    if "nc" not in _NC_CACHE:
        _NC_CACHE["nc"] = _build_nc()
    return _NC_CACHE["nc"]


def _host_prep(x, cos, sin, wq, wk, wv, wo):
    x = np.asarray(x, dtype=np.float32)
    cos = np.asarray(cos, dtype=np.float32)[:T]
    sin = np.asarray(sin, dtype=np.float32)[:T]
    wq = np.asarray(wq, dtype=np.float32)
    wk = np.asarray(wk, dtype=np.float32)
    wv = np.asarray(wv, dtype=np.float32)
    wo = np.asarray(wo, dtype=np.float32)

    xT = np.ascontiguousarray(x.reshape(TOK, DIM).T).astype(BF16_NP)
    woT = np.ascontiguousarray(wo.T).astype(BF16_NP)

    pair = np.arange(64) // 2
    cd = cos[:, pair].T
    sd = sin[:, pair].T
    cdup = np.ascontiguousarray(np.concatenate([cd, cd], axis=0)).astype(BF16_NP)
    sdup = np.ascontiguousarray(np.concatenate([sd, sd], axis=0)).astype(BF16_NP)

    R = np.zeros((64, 64), dtype=np.float32)
    for j in range(32):
        R[2 * j, 2 * j + 1] = -1.0
        R[2 * j + 1, 2 * j] = 1.0
    RT = R.T
    r2t = np.zeros((128, 128), dtype=np.float32)
    r2t[0:64, 0:64] = RT
    r2t[64:128, 64:128] = RT
    rt2 = np.concatenate([RT, RT], axis=1)
    iid = np.concatenate([np.eye(64, dtype=np.float32)] * 2, axis=1)
    ident = np.eye(128, dtype=np.float32)
    kp = np.arange(128)[:, None]
    cc = np.arange(128)[None, :]
    tri = (kp <= cc).astype(np.float32)

    consts = {
        "xT": xT, "woT": woT, "cdup": cdup, "sdup": sdup,
        "r2t": r2t.astype(BF16_NP),
        "rt2": np.ascontiguousarray(rt2).astype(BF16_NP),
        "iid": np.ascontiguousarray(iid).astype(BF16_NP),
        "ident": ident.astype(BF16_NP), "tri": tri.astype(BF16_NP),
        "ones64": np.ones((1, 64), dtype=np.float32).astype(BF16_NP),
        "warm": np.zeros((8, 16), dtype=np.float32).astype(BF16_NP),
    }
    in_maps = []
    for g in range(CORES):
        wqkv = np.concatenate([
            wq[g * 256:(g + 1) * 256],
            wk[g * 64:(g + 1) * 64],
            wv[g * 64:(g + 1) * 64],
        ], axis=0)
        m = dict(consts)
        m["wqkvT"] = np.ascontiguousarray(wqkv.T).astype(BF16_NP)
        in_maps.append(m)
    return in_maps


def run(inputs, trace=False):
    _install_ntff_hook()
    from concourse import bass_utils
    nc = _get_nc()
    in_maps = _host_prep(**inputs)
    res = bass_utils.run_bass_kernel_spmd(
        nc, in_maps, core_ids=list(range(CORES)), trace=trace)
    full = np.empty((TOK, DIM), dtype=np.float32)
    for g in range(CORES):
        full[g * TOK_PER_CORE:(g + 1) * TOK_PER_CORE] = res.results[g]["out"]
    return full.reshape(B, T, DIM), res.exec_time_ns


def kernel(x, cos, sin, wq, wk, wv, wo):
    out, _ = run(dict(x=x, cos=cos, sin=sin, wq=wq, wk=wk, wv=wv, wo=wo),
                 trace=False)
    return out


# revision 15
# speedup vs baseline: 1.4851x; 1.0528x over previous
"""GroupedQueryAttention Trainium2 kernel (8 NeuronCores, tensor-parallel).

Sharding: heads across cores (4 q-heads / 1 kv-head per core).  Per core:
  - q/k/v projections for its heads (bf16 matmuls, fp32 PSUM accum)
  - interleaved RoPE applied via a rotation matmul + elementwise combine
  - causal flash-style attention in transposed layout:
      S^T[k,q] = K^T q  (row-packed pairs of heads, K=64 each),
      P = exp(S/8)  on ACT, row-sums via ones-augmented V (M=65 PV matmul)
  - AllToAll redistributes y from head-sharded to token-sharded
  - output projection for the core's 512-token slice, host concatenates.

Emission order interleaves batch-1 projections/RoPE after batch-0
attention so the PE queue has filler work while ACT runs the softmax
exps; ACT carries only exps, copies run on DVE, causal masks on GpSimd.

B=2, T=2048, DIM=2048, 32 q-heads / 8 kv-heads, head_dim 64.
"""
import os
import sys
import types

os.environ["JAX_PLATFORMS"] = ""  # axon PJRT must be allowed to register
if "/opt/trn_rl_repo" not in sys.path:
    sys.path.insert(0, "/opt/trn_rl_repo")

import numpy as np
import ml_dtypes

BF16_NP = ml_dtypes.bfloat16

B, T, DIM = 2, 2048, 2048
N_HEADS, N_KV_HEADS = 32, 8
HD = 64
CORES = 8
HLOC = N_HEADS // CORES      # 4 local q heads
PAIRS = HLOC // 2            # 2 head pairs
TOK = B * T
TQ = 512
TK = 128
NQT = T // TQ                # 4 q-tiles per batch
NKB = T // TK                # 16 key blocks per batch
TOK_PER_CORE = TOK // CORES  # 512


def _install_ntff_hook():
    import antenv
    if "antenv.axon_hooks" in sys.modules:
        return
    mod = types.ModuleType("antenv.axon_hooks")
    mod._hook = None
    def _set(h): mod._hook = h
    def _get(): return mod._hook
    mod.set_axon_ntff_profile_hook = _set
    mod.get_axon_ntff_profile_hook = _get
    sys.modules["antenv.axon_hooks"] = mod
    antenv.axon_hooks = mod
    try:
        from trn_agent_boot.trn_boot import _ntff_profile_via_ctypes
        _set(_ntff_profile_via_ctypes("/opt/axon/libaxon_pjrt.so"))
    except Exception:
        pass


def _build_nc():
    import concourse.mybir as mybir
    import concourse.tile as tile
    from concourse import bacc

    F32 = mybir.dt.float32
    BF16 = mybir.dt.bfloat16
    EXP = mybir.ActivationFunctionType.Exp

    nc = bacc.Bacc("TRN2", target_bir_lowering=False, debug=False,
                   num_devices=CORES)

    xT = nc.dram_tensor("xT", [DIM, TOK], BF16, kind="ExternalInput")
    wqkvT = nc.dram_tensor("wqkvT", [DIM, 384], BF16, kind="ExternalInput")
    woT = nc.dram_tensor("woT", [DIM, DIM], BF16, kind="ExternalInput")
    cdup = nc.dram_tensor("cdup", [128, T], BF16, kind="ExternalInput")
    sdup = nc.dram_tensor("sdup", [128, T], BF16, kind="ExternalInput")
    r2t = nc.dram_tensor("r2t", [128, 128], BF16, kind="ExternalInput")
    rt2 = nc.dram_tensor("rt2", [64, 128], BF16, kind="ExternalInput")
    iid = nc.dram_tensor("iid", [64, 128], BF16, kind="ExternalInput")
    ident = nc.dram_tensor("ident", [128, 128], BF16, kind="ExternalInput")
    tri = nc.dram_tensor("tri", [128, 128], BF16, kind="ExternalInput")
    ones64 = nc.dram_tensor("ones64", [1, 64], BF16, kind="ExternalInput")
    warm = nc.dram_tensor("warm", [8, 16], BF16, kind="ExternalInput")

    out = nc.dram_tensor("out", [TOK_PER_CORE, DIM], F32, kind="ExternalOutput")
    warm_o = nc.dram_tensor("warm_o", [8, 16], BF16, kind="ExternalOutput")

    rg = [list(range(CORES))]

    with tile.TileContext(nc) as tc:
        with (
            tc.tile_pool(name="res", bufs=1) as res,
            tc.tile_pool(name="pre", bufs=3) as pre,
            tc.tile_pool(name="xtp", bufs=4) as xtp,
            tc.tile_pool(name="sb", bufs=2) as sb,
            tc.tile_pool(name="psb", bufs=2, space="PSUM") as psb,
            tc.tile_pool(name="pss", bufs=4, space="PSUM") as pss,
            tc.tile_pool(name="dram", bufs=1, space="DRAM") as dram,
        ):
            w_sb = res.tile([128, 16 * 384], BF16, name="w_sb")
            cdup_sb = res.tile([128, T], BF16, name="cdup_sb")
            sdup_sb = res.tile([128, T], BF16, name="sdup_sb")
            r2t_sb = res.tile([128, 128], BF16, name="r2t_sb")
            rt2_sb = res.tile([64, 128], BF16, name="rt2_sb")
            iid_sb = res.tile([64, 128], BF16, name="iid_sb")
            ident_sb = res.tile([128, 128], BF16, name="ident_sb")
            tri_sb = res.tile([128, 128], BF16, name="tri_sb")
            ones_sb = res.tile([1, 64], BF16, name="ones_sb")
            qrope = [res.tile([128, TOK], BF16, name=f"qrope{s}") for s in range(2)]
            krope = res.tile([128, TOK], BF16, name="krope")
            vaug = res.tile([128, 2 * NKB * 65], BF16, name="vaug")
            wo_sb = res.tile([128, 16 * 2048], BF16, name="wo_sb")

            qpre = [pre.tile([128, TOK], BF16, name=f"qpre{s}", tag="pre")
                    for s in range(2)]
            kvpre = pre.tile([128, TOK], BF16, name="kvpre", tag="pre")

            warm_in = dram.tile([8, 16], BF16, name="warm_in")
            warm_out = dram.tile([8, 16], BF16, name="warm_out")
            a2a_in = [dram.tile([CORES, 128, TOK_PER_CORE], BF16, name=f"a2a_in{p}")
                      for p in range(PAIRS)]
            a2a_out = [dram.tile([CORES, 128, TOK_PER_CORE], BF16, name=f"a2a_out{p}")
                       for p in range(PAIRS)]

            # --- consts on scalar queue; w_sb slabs stream inside quarter 0
            nc.scalar.dma_start(out=cdup_sb[:], in_=cdup[:, :])
            nc.scalar.dma_start(out=sdup_sb[:], in_=sdup[:, :])
            nc.scalar.dma_start(out=r2t_sb[:], in_=r2t[:, :])
            nc.scalar.dma_start(out=rt2_sb[:], in_=rt2[:, :])
            nc.scalar.dma_start(out=iid_sb[:], in_=iid[:, :])
            nc.scalar.dma_start(out=ident_sb[:], in_=ident[:, :])
            nc.scalar.dma_start(out=tri_sb[:], in_=tri[:, :])
            nc.scalar.dma_start(out=ones_sb[:], in_=ones64[:, :])
            nc.gpsimd.memset(vaug[:], 1.0)
            for i in range(24):   # HAM warm-up: dense PE work from ~2us on
                wup = pss.tile([128, 128], F32, name="wup", tag="sm")
                nc.tensor.matmul(wup[:], ident_sb[:], ident_sb[:])
            # warm-up collective on the gpsimd queue only: absorbs cross-core
            # start skew without ever blocking the sync DMA queue.
            nc.gpsimd.dma_start(out=warm_in[:], in_=warm[:, :])
            nc.gpsimd.collective_compute(
                "AllToAll", mybir.AluOpType.bypass, replica_groups=rg,
                ins=[warm_in.opt()], outs=[warm_out.opt()])
            nc.gpsimd.dma_start(out=warm_o[:, :], in_=warm_out[:])

            def proj_quarter(nq):
                c0 = nq * 1024
                ps_q0 = psb.tile([128, 1024], F32, name="ps_q0", tag="big")
                ps_q1 = psb.tile([128, 1024], F32, name="ps_q1", tag="big")
                ps_kv0 = pss.tile([128, 512], F32, name="ps_kv0", tag="sm")
                ps_kv1 = pss.tile([128, 512], F32, name="ps_kv1", tag="sm")
                for kt in range(16):
                    if nq == 0:
                        nc.sync.dma_start(out=w_sb[:, kt * 384:(kt + 1) * 384],
                                          in_=wqkvT[kt * 128:(kt + 1) * 128, :])
                    xt = xtp.tile([128, 1024], BF16, name="xt", tag="xt")
                    nc.sync.dma_start(
                        out=xt[:], in_=xT[kt * 128:(kt + 1) * 128, c0:c0 + 1024])
                    st, sp = (kt == 0), (kt == 15)
                    w0 = w_sb[:, kt * 384:kt * 384 + 128]
                    w1 = w_sb[:, kt * 384 + 128:kt * 384 + 256]
                    w2 = w_sb[:, kt * 384 + 256:kt * 384 + 384]
                    nc.tensor.matmul(ps_q0[:, 0:512], w0, xt[:, 0:512], start=st, stop=sp)
                    nc.tensor.matmul(ps_q0[:, 512:1024], w0, xt[:, 512:1024], start=st, stop=sp)
                    nc.tensor.matmul(ps_q1[:, 0:512], w1, xt[:, 0:512], start=st, stop=sp)
                    nc.tensor.matmul(ps_q1[:, 512:1024], w1, xt[:, 512:1024], start=st, stop=sp)
                    nc.tensor.matmul(ps_kv0[:], w2, xt[:, 0:512], start=st, stop=sp)
                    nc.tensor.matmul(ps_kv1[:], w2, xt[:, 512:1024], start=st, stop=sp)
                nc.vector.tensor_copy(qpre[0][:, c0:c0 + 1024], ps_q0[:])
                nc.vector.tensor_copy(qpre[1][:, c0:c0 + 1024], ps_q1[:])
                nc.vector.tensor_copy(kvpre[:, c0:c0 + 512], ps_kv0[:])
                nc.vector.tensor_copy(kvpre[:, c0 + 512:c0 + 1024], ps_kv1[:])

            def rope_batch(b):
                for ci in range(4):
                    c = b * 4 + ci
                    for j in range(4):
                        kb = ci * 4 + j
                        col = c * 512 + j * 128
                        vt_ps = pss.tile([128, 64], BF16, name="vt_ps", tag="sm")
                        nc.tensor.transpose(
                            vt_ps[:], kvpre[64:128, col:col + 128],
                            ident_sb[64:128, 64:128])
                        va = (b * NKB + kb) * 65
                        nc.vector.tensor_copy(vaug[:, va:va + 64], vt_ps[:])
                for ci in range(4):
                    c = b * 4 + ci
                    gcols = slice(c * 512, (c + 1) * 512)
                    pcols = slice(ci * 512, (ci + 1) * 512)
                    rot_ps = pss.tile([128, 512], F32, name="rot_ps", tag="sm")
                    kdup_ps = pss.tile([128, 512], F32, name="kdup_ps", tag="sm")
                    nc.tensor.matmul(rot_ps[:], rt2_sb[:], kvpre[0:64, gcols])
                    nc.tensor.matmul(kdup_ps[:], iid_sb[:], kvpre[0:64, gcols])
                    rot_sb = sb.tile([128, 512], BF16, name="rot_sb", tag="rope", bufs=4)
                    kdup_sb = sb.tile([128, 512], BF16, name="kdup_sb", tag="rope", bufs=4)
                    nc.scalar.copy(rot_sb[:], rot_ps[:])
                    nc.scalar.copy(kdup_sb[:], kdup_ps[:])
                    t1k = sb.tile([128, 512], BF16, name="t1k", tag="rope", bufs=4)
                    nc.vector.tensor_mul(t1k[:], kdup_sb[:], cdup_sb[:, pcols])
                    nc.vector.tensor_mul(rot_sb[:], rot_sb[:], sdup_sb[:, pcols])
                    nc.vector.tensor_add(krope[:, gcols], t1k[:], rot_sb[:])
                for s in range(2):
                    for ci in range(4):
                        c = b * 4 + ci
                        gcols = slice(c * 512, (c + 1) * 512)
                        pcols = slice(ci * 512, (ci + 1) * 512)
                        rq_ps = pss.tile([128, 512], F32, name="rq_ps", tag="sm")
                        nc.tensor.matmul(rq_ps[:], r2t_sb[:], qpre[s][:, gcols])
                        rq_sb = sb.tile([128, 512], BF16, name="rq_sb", tag="rope", bufs=4)
                        nc.scalar.copy(rq_sb[:], rq_ps[:])
                        t1q = sb.tile([128, 512], BF16, name="t1q", tag="rope", bufs=4)
                        nc.vector.tensor_mul(t1q[:], qpre[s][:, gcols], cdup_sb[:, pcols])
                        nc.vector.tensor_mul(rq_sb[:], rq_sb[:], sdup_sb[:, pcols])
                        nc.vector.tensor_add(qrope[s][:, gcols], t1q[:], rq_sb[:])

            ep_pend = []   # deferred epilogues, flushed into later qt loops

            def flush_ep():
                while ep_pend:
                    p_, shard, yac_, e = ep_pend.pop(0)
                    lrow = sb.tile([1, 512], F32, name="lrow", tag="lin0", bufs=3)
                    nc.vector.tensor_copy(lrow[:], yac_[64:65, 0:512])
                    linv = sb.tile([1, 512], F32, name="linv", tag="lin", bufs=3)
                    nc.vector.reciprocal_approx_fast(linv[:], lrow[:])
                    linb = sb.tile([1, 512], BF16, name="linb", tag="lin2", bufs=3)
                    nc.vector.tensor_copy(linb[:], linv[:])
                    bc_sb = sb.tile([64, 512], BF16, name="bc_sb", tag="ep", bufs=4)
                    nc.gpsimd.partition_broadcast(bc_sb[:], linb[:])
                    y_sb = sb.tile([64, 512], BF16, name="y_sb", tag="ep", bufs=4)
                    nc.vector.tensor_mul(y_sb[:], yac_[0:64, 0:512], bc_sb[:])
                    nc.sync.dma_start(
                        out=a2a_in[p_][shard, e * 64:(e + 1) * 64, :],
                        in_=y_sb[:])

            def attn(p, b):
                bcol = b * T
                for qt in range(NQT):
                    q0 = bcol + qt * TQ
                    nkb = 4 * qt + 4
                    yac = [pss.tile([65, 512], F32, name=f"yac{e}", tag="sm")
                           for e in range(2)]

                    def do_pv(kb, lo, pt):
                        va = (b * NKB + kb) * 65
                        for e in range(2):
                            nc.tensor.matmul(
                                yac[e][0:65, lo:512],
                                vaug[:, va:va + 65],
                                pt[:, e * 512 + lo:e * 512 + 512],
                                start=(kb == 0), stop=(kb == nkb - 1))

                    pend = None   # PV trails ST by one slot: exp latency hides
                    for kb in range(nkb):
                        r = kb - 4 * qt
                        lo = 128 * r if r >= 0 else 0
                        k0 = bcol + kb * TK
                        st_ps = psb.tile([128, 1024], F32, name="st_ps", tag="big")
                        pt = sb.tile([128, 1024], BF16, name="pt", tag="pt", bufs=3)
                        for e in range(2):
                            nc.tensor.matmul(
                                st_ps[:, e * 512 + lo:e * 512 + 512],
                                krope[64 * e:64 * e + 64, k0:k0 + 128],
                                qrope[p][64 * e:64 * e + 64, q0 + lo:q0 + 512])
                        if pend is not None:
                            do_pv(*pend)
                        if kb == 1:
                            flush_ep()   # prev qt's epilogues, latency hidden
                        if r <= 0:
                            nc.scalar.activation(
                                pt[:, 0:1024], st_ps[:, 0:1024], EXP, scale=0.125)
                            if r == 0:
                                for e in range(2):
                                    nc.vector.tensor_mul(
                                        pt[:, e * 512:e * 512 + 128],
                                        pt[:, e * 512:e * 512 + 128],
                                        tri_sb[:])
                        else:
                            for e in range(2):
                                nc.scalar.activation(
                                    pt[:, e * 512 + lo:e * 512 + 512],
                                    st_ps[:, e * 512 + lo:e * 512 + 512],
                                    EXP, scale=0.125)
                                nc.vector.tensor_mul(
                                    pt[:, e * 512 + lo:e * 512 + lo + 128],
                                    pt[:, e * 512 + lo:e * 512 + lo + 128],
                                    tri_sb[:])
                        pend = (kb, lo, pt)
                    do_pv(*pend)
                    shard = b * NQT + qt
                    for e in range(2):
                        ep_pend.append((p, shard, yac[e], e))

            # ---- emission: b0 pipeline, attention gap-filled by b1 prep ----
            proj_quarter(0)
            proj_quarter(1)
            rope_batch(0)
            attn(0, 0)
            proj_quarter(2)          # fills PE gaps while ACT runs b0 exps
            proj_quarter(3)
            rope_batch(1)
            # wo slabs: stream during attention (scalar queue is idle-ish)
            for kt in range(16):
                nc.scalar.dma_start(out=wo_sb[:, kt * 2048:(kt + 1) * 2048],
                                    in_=woT[kt * 128:(kt + 1) * 128, :])
            attn(0, 1)
            flush_ep()
            nc.gpsimd.collective_compute(
                "AllToAll", mybir.AluOpType.bypass, replica_groups=rg,
                ins=[a2a_in[0].opt()], outs=[a2a_out[0].opt()])
            ysl = [pre.tile([128, 8 * 512], BF16, name=f"ysl{i}", tag="pre")
                   for i in range(2)]
            attn(1, 0)
            for kt in range(0, 16, 2):   # pair-0 slabs (a2a #0 done long ago)
                nc.gpsimd.dma_start(
                    out=ysl[kt // 8][:, (kt % 8) * 512:(kt % 8) * 512 + 512],
                    in_=a2a_out[0][kt // 2, :, :])
            attn(1, 1)
            flush_ep()
            nc.gpsimd.collective_compute(
                "AllToAll", mybir.AluOpType.bypass, replica_groups=rg,
                ins=[a2a_in[1].opt()], outs=[a2a_out[1].opt()])
            for kt in range(1, 16, 2):
                nc.gpsimd.dma_start(
                    out=ysl[kt // 8][:, (kt % 8) * 512:(kt % 8) * 512 + 512],
                    in_=a2a_out[1][kt // 2, :, :])

            # ---- out projection, two passes: even kt (a2a #0 data) runs while
            # a2a #1 is in flight, partials staged in SBUF; odd kt added after.
            parts = []
            for mt in range(4):
                op0 = psb.tile([128, 1024], F32, name="op0", tag="big")
                op1 = psb.tile([128, 1024], F32, name="op1", tag="big")
                for idx, kt in enumerate(range(0, 16, 2)):
                    st, sp = (idx == 0), (idx == 7)
                    col = (kt % 8) * 512 + mt * 128
                    lhs = ysl[kt // 8][:, col:col + 128]
                    wcol = kt * 2048
                    nc.tensor.matmul(op0[:, 0:512], lhs, wo_sb[:, wcol:wcol + 512], start=st, stop=sp)
                    nc.tensor.matmul(op0[:, 512:1024], lhs, wo_sb[:, wcol + 512:wcol + 1024], start=st, stop=sp)
                    nc.tensor.matmul(op1[:, 0:512], lhs, wo_sb[:, wcol + 1024:wcol + 1536], start=st, stop=sp)
                    nc.tensor.matmul(op1[:, 512:1024], lhs, wo_sb[:, wcol + 1536:wcol + 2048], start=st, stop=sp)
                part = sb.tile([128, 2048], BF16, name=f"part{mt}", tag=f"part{mt}", bufs=1)
                nc.vector.tensor_copy(part[:, 0:1024], op0[:])
                nc.vector.tensor_copy(part[:, 1024:2048], op1[:])
                parts.append(part)
            for mt in range(4):
                op0 = psb.tile([128, 1024], F32, name="op0b", tag="big")
                op1 = psb.tile([128, 1024], F32, name="op1b", tag="big")
                for idx, kt in enumerate(range(1, 16, 2)):
                    st, sp = (idx == 0), (idx == 7)
                    col = (kt % 8) * 512 + mt * 128
                    lhs = ysl[kt // 8][:, col:col + 128]
                    wcol = kt * 2048
                    nc.tensor.matmul(op0[:, 0:512], lhs, wo_sb[:, wcol:wcol + 512], start=st, stop=sp)
                    nc.tensor.matmul(op0[:, 512:1024], lhs, wo_sb[:, wcol + 512:wcol + 1024], start=st, stop=sp)
                    nc.tensor.matmul(op1[:, 0:512], lhs, wo_sb[:, wcol + 1024:wcol + 1536], start=st, stop=sp)
                    nc.tensor.matmul(op1[:, 512:1024], lhs, wo_sb[:, wcol + 1536:wcol + 2048], start=st, stop=sp)
                for half, op in ((0, op0), (1, op1)):
                    osb = sb.tile([128, 1024], F32, name="osb", tag="osb", bufs=2)
                    nc.vector.tensor_add(
                        osb[:], op[:], parts[mt][:, half * 1024:(half + 1) * 1024])
                    nc.sync.dma_start(
                        out=out[mt * 128:(mt + 1) * 128, half * 1024:(half + 1) * 1024],
                        in_=osb[:])

    nc.compile()
    return nc


_NC_CACHE = {}


def _get_nc():
    if "nc" not in _NC_CACHE:
        _NC_CACHE["nc"] = _build_nc()
    return _NC_CACHE["nc"]


def _host_prep(x, cos, sin, wq, wk, wv, wo):
    x = np.asarray(x, dtype=np.float32)
    cos = np.asarray(cos, dtype=np.float32)[:T]
    sin = np.asarray(sin, dtype=np.float32)[:T]
    wq = np.asarray(wq, dtype=np.float32)
    wk = np.asarray(wk, dtype=np.float32)
    wv = np.asarray(wv, dtype=np.float32)
    wo = np.asarray(wo, dtype=np.float32)

    xT = np.ascontiguousarray(x.reshape(TOK, DIM).T).astype(BF16_NP)
    woT = np.ascontiguousarray(wo.T).astype(BF16_NP)

    pair = np.arange(64) // 2
    cd = cos[:, pair].T
    sd = sin[:, pair].T
    cdup = np.ascontiguousarray(np.concatenate([cd, cd], axis=0)).astype(BF16_NP)
    sdup = np.ascontiguousarray(np.concatenate([sd, sd], axis=0)).astype(BF16_NP)

    R = np.zeros((64, 64), dtype=np.float32)
    for j in range(32):
        R[2 * j, 2 * j + 1] = -1.0
        R[2 * j + 1, 2 * j] = 1.0
    RT = R.T
    r2t = np.zeros((128, 128), dtype=np.float32)
    r2t[0:64, 0:64] = RT
    r2t[64:128, 64:128] = RT
    rt2 = np.concatenate([RT, RT], axis=1)
    iid = np.concatenate([np.eye(64, dtype=np.float32)] * 2, axis=1)
    ident = np.eye(128, dtype=np.float32)
    kp = np.arange(128)[:, None]
    cc = np.arange(128)[None, :]
    tri = (kp <= cc).astype(np.float32)

    consts = {
        "xT": xT, "woT": woT, "cdup": cdup, "sdup": sdup,
        "r2t": r2t.astype(BF16_NP),
        "rt2": np.ascontiguousarray(rt2).astype(BF16_NP),
        "iid": np.ascontiguousarray(iid).astype(BF16_NP),
        "ident": ident.astype(BF16_NP), "tri": tri.astype(BF16_NP),
        "ones64": np.ones((1, 64), dtype=np.float32).astype(BF16_NP),
        "warm": np.zeros((8, 16), dtype=np.float32).astype(BF16_NP),
    }
    in_maps = []
    for g in range(CORES):
        wqkv = np.concatenate([
            wq[g * 256:(g + 1) * 256],
            wk[g * 64:(g + 1) * 64],
            wv[g * 64:(g + 1) * 64],
        ], axis=0)
        m = dict(consts)
        m["wqkvT"] = np.ascontiguousarray(wqkv.T).astype(BF16_NP)
        in_maps.append(m)
    return in_maps


def run(inputs, trace=False):
    _install_ntff_hook()
    from concourse import bass_utils
    nc = _get_nc()
    in_maps = _host_prep(**inputs)
    res = bass_utils.run_bass_kernel_spmd(
        nc, in_maps, core_ids=list(range(CORES)), trace=trace)
    full = np.empty((TOK, DIM), dtype=np.float32)
    for g in range(CORES):
        full[g * TOK_PER_CORE:(g + 1) * TOK_PER_CORE] = res.results[g]["out"]
    return full.reshape(B, T, DIM), res.exec_time_ns


def kernel(x, cos, sin, wq, wk, wv, wo):
    out, _ = run(dict(x=x, cos=cos, sin=sin, wq=wq, wk=wk, wv=wv, wo=wo),
                 trace=False)
    return out
